# revision 5
# baseline (speedup 1.0000x reference)
"""Multi-scale bilinear warp (grid_sample) kernel for Trainium2, 8 NeuronCores.

Strategy (per core, data-parallel over batch: 2 images/core x 4 scales):
  - Row-pad each ref channel-plane in DRAM ([-8..S+8] rows), zeros outside.
  - For each block of 8x R output rows: build, in SBUF, 12 "tap planes" per
    16-partition Q7 core-group: q = 4c + 2k + l holds the (k,l)-shifted,
    column-padded band of channel c (partition 16g+q, g = core group).
  - One gpsimd ap_gather per block: idx = y0*Wp + x0 (per pixel, shared
    across the 16 partitions of a group) fetches all 12 tap values per pixel.
  - DVE 32x32 stream-transpose puts the 12 taps of each pixel into the free
    dim; a short weighted-sum (bilinear weights from the flow) produces the
    output, DMA'd straight to DRAM.
All index/weight math is exact-in-fp32 rint tricks; zero padding reproduces
grid_sample's zeros+valid-mask semantics exactly.
"""
import sys
sys.path.insert(0, "/opt/trn_rl_repo")
import numpy as np
from contextlib import ExitStack

import concourse.bass as bass
import concourse.tile as tile
from concourse import bacc, mybir

f32 = mybir.dt.float32
i16 = mybir.dt.int16
ADD = mybir.AluOpType.add
SUB = mybir.AluOpType.subtract
MUL = mybir.AluOpType.mult

SCALES = [512, 256, 128, 64]
RPG = {512: 16, 256: 16, 128: 16, 64: 8}   # output rows per group per block
CMAG = float(3 * 2**22)                     # rint constant
PAD = 7                                     # col pad ; ramps carry +6.5 (pad - 0.5)


def _geom(S):
    R = RPG[S]
    Wp = S + 14
    NBLK = S // (8 * R)
    N = R * S            # idxs per group per block
    Q = N // 32
    M = N // 16
    NE = (R + 15) * Wp + 16   # gather num_elems (allocated band elems)
    assert NE <= 32768 and N % 32 == 0 and S % 32 == 0
    return R, Wp, NBLK, N, Q, M, NE


def make_consts():
    cols = []
    for S in SCALES:
        R, Wp, NBLK, N, Q, M, NE = _geom(S)
        p = np.arange(128) % 16
        i32 = np.arange(128) % 32
        u = np.arange(Q)
        for h in (0, 1):
            pix = (p[:, None] + 16 * h) * Q + u[None, :]
            cols.append((pix // S) + 6.5)          # rampRw_h
            cols.append((pix % S) + 6.5)           # rampJw_h
        pixp = i32[:, None] * Q + u[None, :]
        cols.append((pixp // S) + 6.5)             # rampRp
        cols.append((pixp % S) + 6.5)              # rampJp
    return np.concatenate(cols, axis=1).astype(np.float32)


def build_kernel(scales_on=(0, 1, 2, 3)):
    nc = bacc.Bacc("TRN2", target_bir_lowering=False, debug=False)
    consts_np = make_consts()
    CC = consts_np.shape[1]

    refs, flows, outs = {}, {}, {}
    for si, S in enumerate(SCALES):
        refs[si] = nc.dram_tensor(f"ref{si}", (2, 3, S, S), f32, kind="ExternalInput")
        flows[si] = nc.dram_tensor(f"flow{si}", (2, 2, S, S), f32, kind="ExternalInput")
        outs[si] = nc.dram_tensor(f"out{si}", (2, 3, S, S), f32, kind="ExternalOutput")
    consts_d = nc.dram_tensor("consts", (128, CC), f32, kind="ExternalInput")

    with tile.TileContext(nc) as tc:
        with ExitStack() as ctx:
            cpool = ctx.enter_context(tc.tile_pool(name="consts", bufs=1))
            dpool = ctx.enter_context(tc.tile_pool(name="dram", bufs=1, space="DRAM"))
            bpool = ctx.enter_context(tc.tile_pool(name="band", bufs=1))
            gpool = ctx.enter_context(tc.tile_pool(name="gath", bufs=1))
            tpool = ctx.enter_context(tc.tile_pool(name="trans", bufs=1))
            spool = ctx.enter_context(tc.tile_pool(name="small", bufs=2))

            ctile = cpool.tile([128, CC], f32, tag="consts")
            nc.sync.dma_start(ctile[:], consts_d.ap()[:])
            zt = cpool.tile([128, 2048], f32, tag="zero")
            nc.vector.memset(zt[:], 0.0)

            # consts column offsets per scale
            coff = {}
            off = 0
            for si, S in enumerate(SCALES):
                R, Wp, NBLK, N, Q, M, NE = _geom(S)
                coff[si] = off
                off += 6 * Q

            # ---- prep: zero-padded DRAM planes [3, S+17, S+2] per (b, scale) ----
            rp = {}
            ZW = 2048
            for si in scales_on:
                S = SCALES[si]
                PL = (S + 17) * (S + 2)
                for b in range(2):
                    t = dpool.tile([3 * PL], f32, tag=f"rp{si}_{b}")
                    rp[(si, b)] = t
                    # zero-fill whole plane in chunks of 128*ZW
                    total = 3 * PL
                    pos = 0
                    while pos < total:
                        n = min(128 * ZW, total - pos)
                        ch = n // 128
                        n = ch * 128
                        if n == 0:
                            ch, n = 1, 128  # tail: 128 elems
                            if pos + n > total:
                                pos = total - n
                        dst = bass.AP(t.tensor, t[:].offset + pos, [[ch, 128], [1, ch]])
                        nc.sync.dma_start(dst, zt[:, 0:ch])
                        pos += n
                    # data rows per channel: rows [8,8+S) cols [1,S+1)
                    for c in range(3):
                        dst = bass.AP(t.tensor,
                                      t[:].offset + c * PL + 8 * (S + 2) + 1,
                                      [[S + 2, S], [1, S]])
                        nc.sync.dma_start(dst, refs[si].ap()[b, c])

            # ---- main loops ----
            for si in scales_on:
                S = SCALES[si]
                R, Wp, NBLK, N, Q, M, NE = _geom(S)
                cs = float(np.float32((S - 1.0) / S))
                co = coff[si]
                rampRw = [ctile[:, co + 0 * Q:co + 1 * Q], ctile[:, co + 2 * Q:co + 3 * Q]]
                rampJw = [ctile[:, co + 1 * Q:co + 2 * Q], ctile[:, co + 3 * Q:co + 4 * Q]]
                rampRp = ctile[:, co + 4 * Q:co + 5 * Q]
                rampJp = ctile[:, co + 5 * Q:co + 6 * Q]

                for b in range(2):
                    for blk in range(NBLK):
                        r00 = blk * 8 * R       # first output row of block
                        band = bpool.tile([128, 16322], f32, tag="band")

                        # col-pad memsets (zeros around data cols; tail)
                        BW = 16322
                        nc.vector.memset(
                            bass.AP(band.tensor, band[:].offset, [[BW, 128], [Wp, R + 15], [1, 6]]), 0.0)
                        nc.vector.memset(
                            bass.AP(band.tensor, band[:].offset + S + 7, [[BW, 128], [Wp, R + 15], [1, 7]]), 0.0)
                        nc.vector.memset(
                            bass.AP(band.tensor, band[:].offset + (R + 15) * Wp, [[BW, 128], [1, 16]]), 0.0)

                        # band loads: per (g,c,k) one DMA covering l=0,1 partitions
                        plane = rp[(si, b)]
                        PL = (S + 17) * (S + 2)
                        for g in range(8):
                            for c in range(3):
                                for k in (0, 1):
                                    q = 16 * g + 4 * c + 2 * k
                                    row0 = r00 + g * R + 1 + k
                                    src = bass.AP(plane.tensor,
                                                  plane[:].offset + c * PL + row0 * (S + 2),
                                                  [[1, 2], [S + 2, R + 15], [1, S + 1]])
                                    dst = bass.AP(band.tensor,
                                                  band[q:, :].offset + 6,
                                                  [[BW, 2], [Wp, R + 15], [1, S + 1]])
                                    nc.sync.dma_start(dst, src)

                        # flow loads for idx (wrapped halves)
                        idx16 = spool.tile([128, M], i16, tag="idx16")
                        for h in (0, 1):
                            fl0 = spool.tile([128, Q], f32, tag="flw0")
                            fl1 = spool.tile([128, Q], f32, tag="flw1")
                            for ch, fl in ((0, fl0), (1, fl1)):
                                base = (b * 2 + ch) * S * S + r00 * S + h * 16 * Q
                                src = bass.AP(flows[si].ap().tensor, base,
                                              [[R * S, 8], [Q, 16], [1, Q]])
                                nc.sync.dma_start(fl[:], src)
                            yh = spool.tile([128, Q], f32, tag="t_yh")
                            y0 = spool.tile([128, Q], f32, tag="t_y0")
                            xh = spool.tile([128, Q], f32, tag="t_xh")
                            x0 = spool.tile([128, Q], f32, tag="t_x0")
                            idf = spool.tile([128, Q], f32, tag="t_idf")
                            nc.vector.scalar_tensor_tensor(yh[:], fl0[:], cs, rampRw[h], MUL, ADD)
                            nc.vector.tensor_scalar(y0[:], yh[:], CMAG, CMAG, ADD, SUB)
                            nc.vector.scalar_tensor_tensor(xh[:], fl1[:], cs, rampJw[h], MUL, ADD)
                            nc.vector.tensor_scalar(x0[:], xh[:], CMAG, CMAG, ADD, SUB)
                            nc.vector.scalar_tensor_tensor(idf[:], y0[:], float(Wp), x0[:], MUL, ADD)
                            dsti = bass.AP(idx16.tensor, idx16[:].offset + h, [[M, 128], [2, Q]])
                            nc.vector.tensor_copy(dsti, idf[:])

                        # gather + transpose
                        G = gpool.tile([128, 8192], f32, tag="G")
                        nc.gpsimd.ap_gather(G[:, 0:N], band[:, 0:NE], idx16[:],
                                            channels=128, num_elems=NE, d=1, num_idxs=N)
                        T = tpool.tile([128, 8192], f32, tag="T")
                        nc.vector.transpose(T[:, 0:N], G[:, 0:N])

                        # weights + combine per e
                        for e in (0, 1):
                            flp0 = spool.tile([128, Q], f32, tag="flp0")
                            flp1 = spool.tile([128, Q], f32, tag="flp1")
                            for ch, fl in ((0, flp0), (1, flp1)):
                                base = (b * 2 + ch) * S * S + r00 * S + e * R * S
                                src = bass.AP(flows[si].ap().tensor, base,
                                              [[2 * R * S, 4], [Q, 32], [1, Q]])
                                nc.sync.dma_start(fl[:], src)
                            xp = spool.tile([128, Q], f32, tag="t_xp")
                            x0p = spool.tile([128, Q], f32, tag="t_x0p")
                            fgx = spool.tile([128, Q], f32, tag="t_fgx")
                            ux0 = spool.tile([128, Q], f32, tag="t_ux0")
                            ux1 = spool.tile([128, Q], f32, tag="t_ux1")
                            nc.vector.scalar_tensor_tensor(xp[:], flp1[:], cs, rampJp, MUL, ADD)
                            nc.vector.tensor_scalar(x0p[:], xp[:], CMAG, CMAG, ADD, SUB)
                            nc.vector.scalar_tensor_tensor(fgx[:], x0p[:], -1.0, xp[:], MUL, ADD)
                            nc.vector.tensor_scalar(ux1[:], fgx[:], 0.5, None, ADD)
                            nc.vector.tensor_scalar(ux0[:], fgx[:], -1.0, 0.5, MUL, ADD)
                            yp = spool.tile([128, Q], f32, tag="t_yp")
                            y0p = spool.tile([128, Q], f32, tag="t_y0p")
                            fgy = spool.tile([128, Q], f32, tag="t_fgy")
                            uy0 = spool.tile([128, Q], f32, tag="t_uy0")
                            uy1 = spool.tile([128, Q], f32, tag="t_uy1")
                            nc.vector.scalar_tensor_tensor(yp[:], flp0[:], cs, rampRp, MUL, ADD)
                            nc.vector.tensor_scalar(y0p[:], yp[:], CMAG, CMAG, ADD, SUB)
                            nc.vector.scalar_tensor_tensor(fgy[:], y0p[:], -1.0, yp[:], MUL, ADD)
                            nc.vector.tensor_scalar(uy1[:], fgy[:], 0.5, None, ADD)
                            nc.vector.tensor_scalar(uy0[:], fgy[:], -1.0, 0.5, MUL, ADD)
                            W = []
                            for t in range(4):
                                w = spool.tile([128, Q], f32, tag=f"t_w{t}")
                                uy = uy1 if (t >> 1) else uy0
                                ux = ux1 if (t & 1) else ux0
                                nc.vector.tensor_tensor(w[:], uy[:], ux[:], MUL)
                                W.append(w)
                            for c in range(3):
                                j0 = 16 * e + 4 * c
                                acc = spool.tile([128, Q], f32, tag="t_acc")
                                tmp = spool.tile([128, Q], f32, tag="t_tmp")
                                tap = lambda t: bass.AP(T.tensor, T[:].offset + j0 + t,
                                                        [[8192, 128], [32, Q]])
                                nc.vector.tensor_tensor(acc[:], W[0][:], tap(0), MUL)
                                for t in (1, 2, 3):
                                    nc.vector.tensor_tensor(tmp[:], W[t][:], tap(t), MUL)
                                    nc.vector.tensor_tensor(acc[:], acc[:], tmp[:], ADD)
                                base = (b * 3 + c) * S * S + r00 * S + e * R * S
                                dsto = bass.AP(outs[si].ap().tensor, base,
                                               [[2 * R * S, 4], [Q, 32], [1, Q]])
                                nc.scalar.dma_start(dsto, acc[:])
    nc.compile()
    return nc, consts_np


_CACHED = None


def kernel(**inputs):
    global _CACHED
    from runner import make_runner  # replaced by inline copy in final version
    if _CACHED is None:
        nc, consts_np = build_kernel()
        run = make_runner(nc, 8)
        _CACHED = (run, consts_np)
    run, consts_np = _CACHED
    in_maps = []
    for core in range(8):
        m = {"consts": consts_np}
        for si in range(4):
            m[f"ref{si}"] = np.asarray(inputs[f"ref{si}"])[2 * core:2 * core + 2]
            m[f"flow{si}"] = np.asarray(inputs[f"flow{si}"])[2 * core:2 * core + 2]
        in_maps.append(m)
    res, best = run(in_maps, time_iters=0)
    outs = []
    for si in range(4):
        outs.append(np.concatenate([res[c][f"out{si}"] for c in range(8)], axis=0))
    return tuple(outs)


if __name__ == "__main__":
    nc, consts_np = build_kernel()
    print("build ok")
